# revision 9
# baseline (speedup 1.0000x reference)
"""Multihead attention on 8 Trainium2 cores (Bass/Tile).

Sharding: core = (batch b, head-group hg); 2 batches x 4 head-groups,
4 heads per core (head dim 64, local width 256).

Per core (matmul operands bf16, PSUM accumulation fp32):
  qT = (Wq[hg]/8 @ x_q^T)          [256, 2048]   (e' on partitions)
  kT = (Wk[hg]  @ x_k^T)           [256, SK]
  v  = (x_k  @ Wv[hg]^T)           [SK, 256]     (natural layout, + ones col)
  scoresT[sk, sq] = kT^T-slices x qT  (PE, head pairs row-packed)
  probsT = exp(scoresT)            (ACT, PSUM fp32 -> SBUF bf16)
  attnT[d, sq], denom[sq] = v_aug^T @ probsT   (ones column -> denominator)
  attn = attnT / denom   (denoms repartitioned via DRAM to [8,128], exact
                          DVE reciprocal there, DRAM-bounce broadcast, mul)
  out_partial[s, :] = attn^T-chunks x Wo[:, hg]^T   (no bias on device)
Host: out[b] = sum of the 4 head-group partials + bo.
All inputs loaded upfront into persistent SBUF tiles, issue order matching
consumption (bq/bk, xq, wk, xk, wv, xv), all on the SP DMA queue.

Mask handling: the key_padding_mask (and compaction padding) zeroes v rows
AND the ones-column, so masked keys contribute exactly 0 to both the
numerator and the softmax denominator -- identical to softmax(-inf masking).

Compaction: only valid (unmasked) key positions are shipped per batch,
padded to a multiple of 128. SK is derived from the inputs at call time;
kernels are compiled once per SK and cached.
"""

import os
import sys

sys.path.insert(0, "/opt/trn_rl_repo")

import ml_dtypes
import numpy as np

import concourse.bass as bass
import concourse.mybir as mybir
import concourse.tile as tile
from concourse import bacc
from concourse.bass_utils import run_bass_kernel_spmd

B, S, E, H, D = 2, 2048, 1024, 16, 64
N_CORES = 8
HL = H // 4          # 4 heads per core
EL = HL * D          # 256 local embed width
PC = EL // 128       # 2 partition chunks of local heads
ECH = E // 128       # 8 contraction chunks for projections
SGRP = S // 512      # 4 query groups
SQT = S // 128       # 16 query tiles

f32 = mybir.dt.float32
f32r = mybir.dt.float32r
bf16 = mybir.dt.bfloat16
nbf16 = ml_dtypes.bfloat16

_cache: dict[int, object] = {}
LAST_EXEC_NS = None
LAST_RESULTS = None


def _bcast_ap(handle, shape):
    """DRAM AP broadcast along partitions: shape [128, ...dims of handle]."""
    ap = handle[:]
    dims = [[0, shape[0]]]
    # strides for the remaining dims over the flat dram tensor
    sizes = shape[1:]
    stride = 1
    rev = []
    for s in reversed(sizes):
        rev.append([stride, s])
        stride *= s
    dims.extend(reversed(rev))
    return bass.AP(tensor=ap.tensor, offset=0, ap=dims)


def _build(SK: int):
    SKT = SK // 128
    TGS = (SKT + 1) // 2
    kgroups = [(o, min(512, SK - o)) for o in range(0, SK, 512)]

    nc = bacc.Bacc(None, target_bir_lowering=False)

    xqT = nc.dram_tensor("xqT", [E, S], bf16, kind="ExternalInput")
    xkT = nc.dram_tensor("xkT", [E, SK], bf16, kind="ExternalInput")
    xvT = nc.dram_tensor("xvT", [E, SK], bf16, kind="ExternalInput")
    wqT = nc.dram_tensor("wqT", [E, EL], bf16, kind="ExternalInput")
    wkT = nc.dram_tensor("wkT", [E, EL], bf16, kind="ExternalInput")
    wvT = nc.dram_tensor("wvT", [E, EL], bf16, kind="ExternalInput")
    woT = nc.dram_tensor("woT", [EL, E], bf16, kind="ExternalInput")
    bq2 = nc.dram_tensor("bq2", [PC, 128], f32, kind="ExternalInput")
    bk2 = nc.dram_tensor("bk2", [PC, 128], f32, kind="ExternalInput")
    bvA = nc.dram_tensor("bvA", [HL * 65], f32, kind="ExternalInput")
    mk = nc.dram_tensor("mk", [SK], f32, kind="ExternalInput")
    ones = nc.dram_tensor("ones", [64], bf16, kind="ExternalInput")
    out = nc.dram_tensor("out", [S, E], bf16, kind="ExternalOutput")

    with tile.TileContext(nc) as tc, nc.allow_low_precision("fp32r attention"):
        with (
            tc.tile_pool(name="persist", bufs=1) as pp,
            tc.tile_pool(name="xs", bufs=3) as xs,
            tc.tile_pool(name="prb", bufs=2 * TGS + 1) as prb,
            tc.tile_pool(name="rcp", bufs=2) as rcp,
            tc.tile_pool(name="bsb", bufs=2) as bsb,
            tc.tile_pool(name="tmp", bufs=2) as tmp,
            tc.tile_pool(name="osb", bufs=2) as osb,
        ):
            # ---- persistent tiles + constant loads ----
            wq_sb = pp.tile([128, ECH, EL], bf16, name="wq_sb", tag="wq_sb")
            wk_sb = pp.tile([128, ECH, EL], bf16, name="wk_sb", tag="wk_sb")
            wv_sb = pp.tile([128, ECH, EL], bf16, name="wv_sb", tag="wv_sb")
            wo_sb = pp.tile([128, PC, E], bf16, name="wo_sb", tag="wo_sb")
            nc.sync.dma_start(out=wq_sb, in_=wqT[:, :].rearrange("(c p) m -> p c m", p=128))

            bq_sb = pp.tile([128, PC], f32, name="bq_sb", tag="bq_sb")
            bk_sb = pp.tile([128, PC], f32, name="bk_sb", tag="bk_sb")

            bv_sb = pp.tile([128, HL, 65], f32, name="bv_sb", tag="bv_sb")
            nc.sync.dma_start(out=bv_sb, in_=_bcast_ap(bvA, [128, HL, 65]))
            ones_sb = pp.tile([128, 64], bf16, name="ones_sb", tag="ones_sb")
            nc.sync.dma_start(out=ones_sb, in_=_bcast_ap(ones, [128, 64]))
            m_sb = pp.tile([128, SKT], f32, name="m_sb", tag="m_sb")
            nc.sync.dma_start(out=m_sb, in_=mk[:].rearrange("(t p) -> p t", p=128))

            # upfront input loads: persistent chunk tiles, split across the
            # two hardware DMA queues (SP + Activation)
            xq_sb = pp.tile([128, ECH, S], bf16, name="xq_sb", tag="xq_sb")
            xk_sb = pp.tile([128, ECH, SK], bf16, name="xk_sb", tag="xk_sb")
            xv_sb = pp.tile([128, ECH, SK], bf16, name="xv_sb", tag="xv_sb")
            xqr = xqT[:, :].rearrange("(c p) s -> c p s", p=128)
            xkr = xkT[:, :].rearrange("(c p) s -> c p s", p=128)
            xvr = xvT[:, :].rearrange("(c p) s -> c p s", p=128)
            nc.sync.dma_start(out=bq_sb, in_=bq2[:, :].rearrange("c p -> p c"))
            nc.sync.dma_start(out=bk_sb, in_=bk2[:, :].rearrange("c p -> p c"))
            for ec in range(ECH):
                nc.sync.dma_start(out=xq_sb[:, ec, :], in_=xqr[ec])
            nc.sync.dma_start(
                out=wk_sb, in_=wkT[:, :].rearrange("(c p) m -> p c m", p=128)
            )
            for ec in range(ECH):
                nc.sync.dma_start(out=xk_sb[:, ec, :], in_=xkr[ec])
            nc.sync.dma_start(
                out=wv_sb, in_=wvT[:, :].rearrange("(c p) m -> p c m", p=128)
            )
            for ec in range(ECH):
                nc.sync.dma_start(out=xv_sb[:, ec, :], in_=xvr[ec])

            qT_sb = [pp.tile([128, S], bf16, name=f"qT{c}", tag=f"qT{c}") for c in range(PC)]
            kT_sb = [pp.tile([128, SK], bf16, name=f"kT{c}", tag=f"kT{c}") for c in range(PC)]
            aT_sb = [pp.tile([128, S], bf16, name=f"aT{c}", tag=f"aT{c}") for c in range(PC)]
            v_sb = [
                pp.tile([128, HL, 65], bf16, name=f"v{t}", tag=f"v{t}") for t in range(SKT)
            ]

            # ---- phase 1: projections ----
            with tc.tile_pool(name="pj", bufs=1, space="PSUM") as pj:
                # q and k: out layout [e' partition, seq free]
                for (xh, slen, glist, wsb, bias_sb, dst) in (
                    (xqT, S, [(o, 512) for o in range(0, S, 512)], wq_sb, bq_sb, qT_sb),
                    (xkT, SK, kgroups, wk_sb, bk_sb, kT_sb),
                ):
                    xsb = xq_sb if xh is xqT else xk_sb
                    pqt = {}
                    for ec in range(ECH):
                        xc = xsb[:, ec, :]
                        for pc in range(PC):
                            for gi, (go, gs) in enumerate(glist):
                                idx = pc * 4 + gi
                                if ec == 0:
                                    pqt[idx] = pj.tile(
                                        [128, 512], f32, name=f"pj{idx}", tag=f"pj{idx}"
                                    )
                                nc.tensor.matmul(
                                    pqt[idx][:, 0:gs],
                                    wsb[:, ec, pc * 128 : pc * 128 + 128],
                                    xc[:, go : go + gs],
                                    start=(ec == 0),
                                    stop=(ec == ECH - 1),
                                )
                    for pc in range(PC):
                        for gi, (go, gs) in enumerate(glist):
                            idx = pc * 4 + gi
                            nc.vector.tensor_scalar_add(
                                out=dst[pc][:, go : go + gs],
                                in0=pqt[idx][:, 0:gs],
                                scalar1=bias_sb[:, pc : pc + 1],
                            )


            nc.sync.dma_start(
                out=wo_sb, in_=woT[:, :].rearrange("(c p) m -> p c m", p=128)
            )
            # ---- phase 2: software-pipelined attention ----
            # scores+exp(g,c) emitted one stage AHEAD of attnV+norm(g,c);
            # the v-projection (which waits on the xv DMA tail) is emitted
            # after the first two scores blocks so the DMA hides under them.
            with (
                tc.tile_pool(name="scr", bufs=1, space="PSUM") as scr,
                tc.tile_pool(name="att", bufs=2, space="PSUM") as att,
                tc.tile_pool(name="rdr", bufs=2, space="DRAM") as rdr,
            ):
                def emit_scores_tg(g, c, tg, pts):
                    gsl = slice(g * 512, (g + 1) * 512)
                    if True:
                        tl = [t for t in (2 * tg, 2 * tg + 1) if t < SKT]
                        nt = len(tl)
                        # one 4-bank tile, slices laid out (i, h2) so a partial
                        # tg (nt=1) stays contiguous for the single exp
                        st = scr.tile([128, 4, 512], f32, name=f"sc_{c}{g}{tg}", tag="sc")
                        for i, t in enumerate(tl):
                            for h2 in range(2):
                                hsl = slice(h2 * 64, (h2 + 1) * 64)
                                nc.tensor.matmul(
                                    st[:, i * 2 + h2, :],
                                    kT_sb[c][hsl, t * 128 : (t + 1) * 128],
                                    qT_sb[c][hsl, gsl],
                                    start=True,
                                    stop=True,
                                )
                        p_ = prb.tile(
                            [128, 4, 512], bf16, name=f"pb_{c}{g}{tg}", tag="pb"
                        )
                        nc.scalar.activation(
                            out=p_[:, 0 : 2 * nt, :],
                            in_=st[:, 0 : 2 * nt, :],
                            func=mybir.ActivationFunctionType.Exp,
                        )
                        pts.append((tl, p_))

                def emit_vproj_pass(t0):
                    # v-projection pass (2 k-tiles) in the idle att PSUM slots
                    tl = list(range(t0, min(t0 + 2, SKT)))
                    if True:
                        pvt = {}
                        for ec in range(ECH):
                            xc = xv_sb[:, ec, t0 * 128 : t0 * 128 + len(tl) * 128]
                            for j, t in enumerate(tl):
                                if ec == 0:
                                    pvt[j] = att.tile(
                                        [128, EL], f32, name=f"pv{t}", tag=f"at{j}"
                                    )
                                nc.tensor.matmul(
                                    pvt[j][:, :],
                                    xc[:, j * 128 : (j + 1) * 128],
                                    wv_sb[:, ec, :],
                                    start=(ec == 0),
                                    stop=(ec == ECH - 1),
                                )
                        for j, t in enumerate(tl):
                            pv_view = pvt[j][:, :].rearrange("p (h d) -> p h d", h=HL)
                            vt = xs.tile([128, HL, 65], f32, name=f"vt{t}", tag="vtmp")
                            nc.vector.tensor_add(
                                out=vt[:, :, 0:64], in0=pv_view, in1=bv_sb[:, :, 0:64]
                            )
                            nc.vector.tensor_copy(
                                out=vt[:, :, 64:65], in_=bv_sb[:, :, 64:65]
                            )
                            nc.vector.tensor_scalar_mul(
                                out=v_sb[t][:, :, :],
                                in0=vt[:, :, :],
                                scalar1=m_sb[:, t : t + 1],
                            )

                def emit_attnv_tg(g, c, pts, tg, at):
                    tl, p_ = pts[tg]
                    for i, t in enumerate(tl):
                        for h2 in range(2):
                            nc.tensor.matmul(
                                at[h2][:, :],
                                v_sb[t][:, 2 * c + h2, :],
                                p_[:, i * 2 + h2, :],
                                start=(t == 0),
                                stop=(t == SKT - 1),
                            )

                def emit_norm(g, c, at):
                    gsl = slice(g * 512, (g + 1) * 512)
                    # normalize: copy denom rows out of PSUM, repartition via
                    # DRAM to [8,128], exact reciprocal there (full-width DVE)
                    rc = rcp.tile([128, 2, 512], f32, name=f"rc_{c}{g}", tag="rc")
                    nc.vector.tensor_copy(out=rc[64:65, 0, :], in_=at[0][64:65, :])
                    nc.vector.tensor_copy(out=rc[64:65, 1, :], in_=at[1][64:65, :])
                    dd = rdr.tile([2, 512], f32, name=f"dd_{c}{g}", tag="dd")
                    nc.sync.dma_start(out=dd[0:1, :], in_=rc[64:65, 0, :])
                    nc.sync.dma_start(out=dd[1:2, :], in_=rc[64:65, 1, :])
                    d8 = tmp.tile([8, 128], f32, name=f"d8_{c}{g}", tag="d8")
                    nc.sync.dma_start(
                        out=d8[:, :], in_=dd[:, :].rearrange("h (a b) -> (h a) b", b=128)
                    )
                    r8 = tmp.tile([8, 128], f32, name=f"r8_{c}{g}", tag="r8")
                    nc.vector.reciprocal(out=r8[:, :], in_=d8[:, :])
                    rd = rdr.tile([2, 512], f32, name=f"rd_{c}{g}", tag="rd")
                    nc.sync.dma_start(
                        out=rd[:, :].rearrange("h (a b) -> (h a) b", b=128), in_=r8[:, :]
                    )
                    bs = bsb.tile([64, 2, 512], f32, name=f"bs_{c}{g}", tag="bs")
                    for h2 in range(2):
                        rsrc = rd[h2 : h2 + 1, :]
                        bc_ap = bass.AP(
                            tensor=rsrc.tensor,
                            offset=rsrc.offset,
                            ap=[[0, 64]] + [list(d) for d in rsrc.ap[1:]],
                        )
                        nc.sync.dma_start(out=bs[:, h2, :], in_=bc_ap)
                    nc.vector.tensor_mul(
                        out=aT_sb[c][0:64, gsl], in0=at[0][0:64, :], in1=bs[:, 0, :]
                    )
                    tb = tmp.tile([64, 512], bf16, name=f"tb{c}{g}", tag="tb")
                    nc.vector.tensor_mul(out=tb, in0=at[1][0:64, :], in1=bs[:, 1, :])
                    nc.sync.dma_start(out=aT_sb[c][64:128, gsl], in_=tb)

                def emit_outproj(sl):
                    ssl = slice(sl * 128, (sl + 1) * 128)
                    pot = scr.tile([128, 2, 512], f32, name=f"po{sl}", tag="sc")
                    for c in range(PC):
                        for jg in range(2):
                            nc.tensor.matmul(
                                pot[:, jg, :],
                                aT_sb[c][:, ssl],
                                wo_sb[:, c, jg * 512 : (jg + 1) * 512],
                                start=(c == 0),
                                stop=(c == PC - 1),
                            )
                    ot = osb.tile([128, E], bf16, name=f"ot{sl}", tag="ot")
                    if sl % 2 == 0:
                        nc.scalar.activation(
                            out=ot[:, :].rearrange("p (j e) -> p j e", j=2),
                            in_=pot[:, :, :],
                            func=mybir.ActivationFunctionType.Identity,
                        )
                    else:
                        nc.vector.tensor_copy(
                            out=ot[:, :].rearrange("p (j e) -> p j e", j=2),
                            in_=pot[:, :, :],
                        )
                    nc.sync.dma_start(out=out[ssl, :], in_=ot)

                def new_at(g, c):
                    return [
                        att.tile([65, 512], f32, name=f"at{h2}_{c}{g}", tag=f"at{h2}")
                        for h2 in range(2)
                    ]

                pending = []  # [(g, c, pts, at)]
                next_sl = 0   # out-proj tiles emitted as pipeline fills
                stage = 0
                for g in range(SGRP):
                    for c in range(PC):
                        pts = []
                        fill_v = g == 0 and c == 1
                        fill = pending[0] if (pending and not fill_v) else None
                        if fill and fill[3] is None:
                            fill = pending[0] = (fill[0], fill[1], fill[2], new_at(fill[0], fill[1]))
                        # aT for group gp is complete after norm(gp, c=1), which
                        # is emitted at the end of stage 2*gp+2 -> fill from
                        # stage 2*gp+3 on (up to 2 sl per stage)
                        sl_quota = 2 if stage >= 4 else 0
                        sl_limit = 4 * max(0, (stage - 2) // 2)
                        for tg in range(TGS):
                            emit_scores_tg(g, c, tg, pts)
                            if fill_v:
                                emit_vproj_pass(2 * tg)
                            elif fill:
                                emit_attnv_tg(fill[0], fill[1], fill[2], tg, fill[3])
                            if sl_quota > 0 and next_sl < sl_limit and tg >= 1:
                                emit_outproj(next_sl)
                                next_sl += 1
                                sl_quota -= 1
                        if fill:
                            pending.pop(0)
                            emit_norm(fill[0], fill[1], fill[3])
                        pending.append((g, c, pts, None))
                        stage += 1
                # drain
                for (g, c, pts, at) in pending:
                    if at is None:
                        at = new_at(g, c)
                    for tg in range(TGS):
                        emit_attnv_tg(g, c, pts, tg, at)
                    emit_norm(g, c, at)
                while next_sl < SQT:
                    emit_outproj(next_sl)
                    next_sl += 1



    nc.finalize()
    return nc


def _get(SK: int):
    if SK not in _cache:
        _cache[SK] = _build(SK)
    return _cache[SK]


def kernel(**inputs) -> np.ndarray:
    global LAST_EXEC_NS, LAST_RESULTS

    q = np.asarray(inputs["query"], dtype=np.float32)
    k = np.asarray(inputs["key"], dtype=np.float32)
    v = np.asarray(inputs["value"], dtype=np.float32)
    kpm = np.asarray(inputs["key_padding_mask"]).astype(bool)
    Wq = np.asarray(inputs["Wq"], dtype=np.float32)
    bq = np.asarray(inputs["bq"], dtype=np.float32)
    Wk = np.asarray(inputs["Wk"], dtype=np.float32)
    bk = np.asarray(inputs["bk"], dtype=np.float32)
    Wv = np.asarray(inputs["Wv"], dtype=np.float32)
    bv = np.asarray(inputs["bv"], dtype=np.float32)
    Wo = np.asarray(inputs["Wo"], dtype=np.float32)
    bo = np.asarray(inputs["bo"], dtype=np.float32)

    compact = not os.environ.get("KERNEL_NO_COMPACT")
    if compact:
        valid = [np.nonzero(~kpm[b])[0] for b in range(B)]
        nv = max(len(ix) for ix in valid)
        SK = max(128, ((nv + 127) // 128) * 128)
        if SK > S:
            SK = S
            compact = False
    if not compact:
        SK = S
        valid = [np.arange(S) for _ in range(B)]

    nc = _get(SK)

    # per-batch tensors
    per_b = []
    for b in range(B):
        ix = valid[b]
        n = len(ix)
        xqT = np.ascontiguousarray(q[b].T).astype(nbf16)
        kc = np.zeros((SK, E), dtype=np.float32)
        vc = np.zeros((SK, E), dtype=np.float32)
        kc[:n] = k[b][ix]
        vc[:n] = v[b][ix]
        xkT = np.ascontiguousarray(kc.T).astype(nbf16)
        xvT = np.ascontiguousarray(vc.T).astype(nbf16)
        mv = np.zeros(SK, dtype=np.float32)
        if compact:
            mv[:n] = 1.0
        else:
            mv[:] = (~kpm[b]).astype(np.float32)
        per_b.append((xqT, xkT, xvT, mv))

    ones = np.ones(64, dtype=nbf16)
    in_maps = []
    for cid in range(N_CORES):
        b, hg = cid // 4, cid % 4
        hsl = slice(hg * EL, (hg + 1) * EL)
        xqT, xkT, xvT, mv = per_b[b]
        bvh = bv[hsl].reshape(HL, 64)
        bvA = np.concatenate([bvh, np.ones((HL, 1), np.float32)], axis=1).ravel()
        in_maps.append(
            {
                "xqT": xqT,
                "xkT": xkT,
                "xvT": xvT,
                "wqT": np.ascontiguousarray((Wq[hsl] / 8.0).T).astype(nbf16),
                "wkT": np.ascontiguousarray(Wk[hsl].T).astype(nbf16),
                "wvT": np.ascontiguousarray(Wv[hsl].T).astype(nbf16),
                "woT": np.ascontiguousarray(Wo[:, hsl].T).astype(nbf16),
                "bq2": (bq[hsl] / 8.0).reshape(PC, 128),
                "bk2": bk[hsl].reshape(PC, 128),
                "bvA": bvA,
                "mk": mv,
                "ones": ones,
            }
        )

    trace = bool(os.environ.get("KERNEL_TRACE"))
    res = run_bass_kernel_spmd(
        nc, in_maps, core_ids=list(range(N_CORES)), trace=trace
    )
    LAST_EXEC_NS = res.exec_time_ns
    LAST_RESULTS = res

    out = np.empty((B, S, E), dtype=np.float32)
    for b in range(B):
        acc = res.results[b * 4]["out"].astype(np.float32)
        for hg in range(1, 4):
            acc = acc + res.results[b * 4 + hg]["out"].astype(np.float32)
        out[b] = acc + bo
    return out



# revision 10
# speedup vs baseline: 1.0799x; 1.0799x over previous
"""Multihead attention on 8 Trainium2 cores (Bass/Tile).

Sharding: core = (batch b, head-group hg); 2 batches x 4 head-groups,
4 heads per core (head dim 64, local width 256).

Per core (matmul operands bf16, PSUM accumulation fp32):
  qT = (Wq[hg]/8 @ x_q^T)          [256, 2048]   (e' on partitions)
  kT = (Wk[hg]  @ x_k^T)           [256, SK]
  v  = (x_k  @ Wv[hg]^T)           [SK, 256]     (natural layout, + ones col)
  scoresT[sk, sq] = kT^T-slices x qT  (PE, head pairs row-packed)
  probsT = exp(scoresT)            (ACT, PSUM fp32 -> SBUF bf16)
  attnT[d, sq], denom[sq] = v_aug^T @ probsT   (ones column -> denominator)
  attn = attnT / denom   (denoms repartitioned via DRAM to [8,128], exact
                          DVE reciprocal there, DRAM-bounce broadcast, mul)
  out_partial[s, :] = attn^T-chunks x Wo[:, hg]^T   (no bias on device)
Host: out[b] = sum of the 4 head-group partials + bo.
All inputs loaded upfront into persistent SBUF tiles, issue order matching
consumption (bq/bk, xq, wk, xk, wv, xv), all on the SP DMA queue.

Mask handling: the key_padding_mask (and compaction padding) zeroes v rows
AND the ones-column, so masked keys contribute exactly 0 to both the
numerator and the softmax denominator -- identical to softmax(-inf masking).

Compaction: only valid (unmasked) key positions are shipped per batch,
padded to a multiple of 128. SK is derived from the inputs at call time;
kernels are compiled once per SK and cached.
"""

import os
import sys

sys.path.insert(0, "/opt/trn_rl_repo")

import ml_dtypes
import numpy as np

import concourse.bass as bass
import concourse.mybir as mybir
import concourse.tile as tile
from concourse import bacc
from concourse.bass_utils import run_bass_kernel_spmd

B, S, E, H, D = 2, 2048, 1024, 16, 64
N_CORES = 8
HL = H // 4          # 4 heads per core
EL = HL * D          # 256 local embed width
PC = EL // 128       # 2 partition chunks of local heads
ECH = E // 128       # 8 contraction chunks for projections
SGRP = S // 512      # 4 query groups
SQT = S // 128       # 16 query tiles

f32 = mybir.dt.float32
f32r = mybir.dt.float32r
bf16 = mybir.dt.bfloat16
nbf16 = ml_dtypes.bfloat16

_cache: dict[int, object] = {}
LAST_EXEC_NS = None
LAST_RESULTS = None


def _bcast_ap(handle, shape):
    """DRAM AP broadcast along partitions: shape [128, ...dims of handle]."""
    ap = handle[:]
    dims = [[0, shape[0]]]
    # strides for the remaining dims over the flat dram tensor
    sizes = shape[1:]
    stride = 1
    rev = []
    for s in reversed(sizes):
        rev.append([stride, s])
        stride *= s
    dims.extend(reversed(rev))
    return bass.AP(tensor=ap.tensor, offset=0, ap=dims)


def _build(SK: int):
    SKT = SK // 128
    TGS = (SKT + 1) // 2
    kgroups = [(o, min(512, SK - o)) for o in range(0, SK, 512)]

    nc = bacc.Bacc(None, target_bir_lowering=False)

    xqT = nc.dram_tensor("xqT", [E, S], bf16, kind="ExternalInput")
    xkT = nc.dram_tensor("xkT", [E, SK], bf16, kind="ExternalInput")
    xvT = nc.dram_tensor("xvT", [E, SK], bf16, kind="ExternalInput")
    wqT = nc.dram_tensor("wqT", [E, EL], bf16, kind="ExternalInput")
    wkT = nc.dram_tensor("wkT", [E, EL], bf16, kind="ExternalInput")
    wvT = nc.dram_tensor("wvT", [E, EL], bf16, kind="ExternalInput")
    woT = nc.dram_tensor("woT", [EL, E], bf16, kind="ExternalInput")
    bq2 = nc.dram_tensor("bq2", [PC, 128], f32, kind="ExternalInput")
    bk2 = nc.dram_tensor("bk2", [PC, 128], f32, kind="ExternalInput")
    bvA = nc.dram_tensor("bvA", [HL * 65], f32, kind="ExternalInput")
    mk = nc.dram_tensor("mk", [SK], f32, kind="ExternalInput")
    ones = nc.dram_tensor("ones", [64], bf16, kind="ExternalInput")
    out = nc.dram_tensor("out", [S, E], bf16, kind="ExternalOutput")

    with tile.TileContext(nc) as tc, nc.allow_low_precision("fp32r attention"):
        with (
            tc.tile_pool(name="persist", bufs=1) as pp,
            tc.tile_pool(name="xs", bufs=3) as xs,
            tc.tile_pool(name="prb", bufs=3 * TGS) as prb,
            tc.tile_pool(name="rcp", bufs=2) as rcp,
            tc.tile_pool(name="bsb", bufs=2) as bsb,
            tc.tile_pool(name="tmp", bufs=2) as tmp,
            tc.tile_pool(name="osb", bufs=2) as osb,
        ):
            # ---- persistent tiles + constant loads ----
            wq_sb = pp.tile([128, ECH, EL], bf16, name="wq_sb", tag="wq_sb")
            wk_sb = pp.tile([128, ECH, EL], bf16, name="wk_sb", tag="wk_sb")
            wv_sb = pp.tile([128, ECH, EL], bf16, name="wv_sb", tag="wv_sb")
            wo_sb = pp.tile([128, PC, E], bf16, name="wo_sb", tag="wo_sb")
            nc.sync.dma_start(out=wq_sb, in_=wqT[:, :].rearrange("(c p) m -> p c m", p=128))

            bq_sb = pp.tile([128, PC], f32, name="bq_sb", tag="bq_sb")
            bk_sb = pp.tile([128, PC], f32, name="bk_sb", tag="bk_sb")

            bv_sb = pp.tile([128, HL, 65], f32, name="bv_sb", tag="bv_sb")
            nc.sync.dma_start(out=bv_sb, in_=_bcast_ap(bvA, [128, HL, 65]))
            ones_sb = pp.tile([128, 64], bf16, name="ones_sb", tag="ones_sb")
            nc.sync.dma_start(out=ones_sb, in_=_bcast_ap(ones, [128, 64]))
            m_sb = pp.tile([128, SKT], f32, name="m_sb", tag="m_sb")
            nc.sync.dma_start(out=m_sb, in_=mk[:].rearrange("(t p) -> p t", p=128))

            # upfront input loads: persistent chunk tiles, split across the
            # two hardware DMA queues (SP + Activation)
            xq_sb = pp.tile([128, ECH, S], bf16, name="xq_sb", tag="xq_sb")
            xk_sb = pp.tile([128, ECH, SK], bf16, name="xk_sb", tag="xk_sb")
            xv_sb = pp.tile([128, ECH, SK], bf16, name="xv_sb", tag="xv_sb")
            xqr = xqT[:, :].rearrange("(c p) s -> c p s", p=128)
            xkr = xkT[:, :].rearrange("(c p) s -> c p s", p=128)
            xvr = xvT[:, :].rearrange("(c p) s -> c p s", p=128)
            nc.sync.dma_start(out=bq_sb, in_=bq2[:, :].rearrange("c p -> p c"))
            nc.sync.dma_start(out=bk_sb, in_=bk2[:, :].rearrange("c p -> p c"))
            for ec in range(ECH):
                nc.sync.dma_start(out=xq_sb[:, ec, :], in_=xqr[ec])
            nc.sync.dma_start(
                out=wk_sb, in_=wkT[:, :].rearrange("(c p) m -> p c m", p=128)
            )
            for ec in range(ECH):
                nc.sync.dma_start(out=xk_sb[:, ec, :], in_=xkr[ec])
            nc.sync.dma_start(
                out=wv_sb, in_=wvT[:, :].rearrange("(c p) m -> p c m", p=128)
            )
            for ec in range(ECH):
                nc.sync.dma_start(out=xv_sb[:, ec, :], in_=xvr[ec])

            qT_sb = [pp.tile([128, S], bf16, name=f"qT{c}", tag=f"qT{c}") for c in range(PC)]
            kT_sb = [pp.tile([128, SK], bf16, name=f"kT{c}", tag=f"kT{c}") for c in range(PC)]
            aT_sb = [pp.tile([128, S], bf16, name=f"aT{c}", tag=f"aT{c}") for c in range(PC)]
            v_sb = [
                pp.tile([128, HL, 65], bf16, name=f"v{t}", tag=f"v{t}") for t in range(SKT)
            ]

            # ---- phase 1: projections ----
            with tc.tile_pool(name="pj", bufs=1, space="PSUM") as pj:
                # q and k: out layout [e' partition, seq free]
                for (xh, slen, glist, wsb, bias_sb, dst) in (
                    (xqT, S, [(o, 512) for o in range(0, S, 512)], wq_sb, bq_sb, qT_sb),
                    (xkT, SK, kgroups, wk_sb, bk_sb, kT_sb),
                ):
                    xsb = xq_sb if xh is xqT else xk_sb
                    pqt = {}
                    for ec in range(ECH):
                        xc = xsb[:, ec, :]
                        for pc in range(PC):
                            for gi, (go, gs) in enumerate(glist):
                                idx = pc * 4 + gi
                                if ec == 0:
                                    pqt[idx] = pj.tile(
                                        [128, 512], f32, name=f"pj{idx}", tag=f"pj{idx}"
                                    )
                                nc.tensor.matmul(
                                    pqt[idx][:, 0:gs],
                                    wsb[:, ec, pc * 128 : pc * 128 + 128],
                                    xc[:, go : go + gs],
                                    start=(ec == 0),
                                    stop=(ec == ECH - 1),
                                )
                    for pc in range(PC):
                        for gi, (go, gs) in enumerate(glist):
                            idx = pc * 4 + gi
                            nc.vector.tensor_scalar_add(
                                out=dst[pc][:, go : go + gs],
                                in0=pqt[idx][:, 0:gs],
                                scalar1=bias_sb[:, pc : pc + 1],
                            )


            nc.sync.dma_start(
                out=wo_sb, in_=woT[:, :].rearrange("(c p) m -> p c m", p=128)
            )
            # ---- phase 2: software-pipelined attention ----
            # scores+exp(g,c) emitted one stage AHEAD of attnV+norm(g,c);
            # the v-projection (which waits on the xv DMA tail) is emitted
            # after the first two scores blocks so the DMA hides under them.
            with (
                tc.tile_pool(name="scr", bufs=1, space="PSUM") as scr,
                tc.tile_pool(name="att", bufs=2, space="PSUM") as att,
                tc.tile_pool(name="rdr", bufs=2, space="DRAM") as rdr,
            ):
                def emit_scores_tg(g, c, tg, pts):
                    gsl = slice(g * 512, (g + 1) * 512)
                    if True:
                        tl = [t for t in (2 * tg, 2 * tg + 1) if t < SKT]
                        nt = len(tl)
                        st = [
                            scr.tile(
                                [128, 2, 512], f32, name=f"sc{h2}_{c}{g}{tg}", tag=f"sc{h2}"
                            )
                            for h2 in range(2)
                        ]
                        for i, t in enumerate(tl):
                            for h2 in range(2):
                                hsl = slice(h2 * 64, (h2 + 1) * 64)
                                nc.tensor.matmul(
                                    st[h2][:, i, :],
                                    kT_sb[c][hsl, t * 128 : (t + 1) * 128],
                                    qT_sb[c][hsl, gsl],
                                    start=True,
                                    stop=True,
                                )
                        pt = []
                        for h2 in range(2):
                            p_ = prb.tile(
                                [128, 2, 512], bf16, name=f"pb{h2}_{c}{g}{tg}", tag=f"pb{h2}"
                            )
                            nc.scalar.activation(
                                out=p_[:, 0:nt, :],
                                in_=st[h2][:, 0:nt, :],
                                func=mybir.ActivationFunctionType.Exp,
                            )
                            pt.append(p_)
                        pts.append((tl, pt))

                def emit_vproj_pass(t0):
                    # v-projection pass (2 k-tiles) in the idle att PSUM slots
                    tl = list(range(t0, min(t0 + 2, SKT)))
                    if True:
                        pvt = {}
                        for ec in range(ECH):
                            xc = xv_sb[:, ec, t0 * 128 : t0 * 128 + len(tl) * 128]
                            for j, t in enumerate(tl):
                                if ec == 0:
                                    pvt[j] = att.tile(
                                        [128, EL], f32, name=f"pv{t}", tag=f"at{j}"
                                    )
                                nc.tensor.matmul(
                                    pvt[j][:, :],
                                    xc[:, j * 128 : (j + 1) * 128],
                                    wv_sb[:, ec, :],
                                    start=(ec == 0),
                                    stop=(ec == ECH - 1),
                                )
                        for j, t in enumerate(tl):
                            pv_view = pvt[j][:, :].rearrange("p (h d) -> p h d", h=HL)
                            vt = xs.tile([128, HL, 65], f32, name=f"vt{t}", tag="vtmp")
                            nc.vector.tensor_add(
                                out=vt[:, :, 0:64], in0=pv_view, in1=bv_sb[:, :, 0:64]
                            )
                            nc.vector.tensor_copy(
                                out=vt[:, :, 64:65], in_=bv_sb[:, :, 64:65]
                            )
                            nc.vector.tensor_scalar_mul(
                                out=v_sb[t][:, :, :],
                                in0=vt[:, :, :],
                                scalar1=m_sb[:, t : t + 1],
                            )

                def emit_attnv_tg(g, c, pts, tg, at):
                    tl, pt = pts[tg]
                    for i, t in enumerate(tl):
                        for h2 in range(2):
                            nc.tensor.matmul(
                                at[h2][:, :],
                                v_sb[t][:, 2 * c + h2, :],
                                pt[h2][:, i, :],
                                start=(t == 0),
                                stop=(t == SKT - 1),
                            )

                def emit_norm(g, c, at):
                    gsl = slice(g * 512, (g + 1) * 512)
                    # normalize: copy denom rows out of PSUM, repartition via
                    # DRAM to [8,128], exact reciprocal there (full-width DVE)
                    rc = rcp.tile([128, 2, 512], f32, name=f"rc_{c}{g}", tag="rc")
                    nc.vector.tensor_copy(out=rc[64:65, 0, :], in_=at[0][64:65, :])
                    nc.vector.tensor_copy(out=rc[64:65, 1, :], in_=at[1][64:65, :])
                    dd = rdr.tile([2, 512], f32, name=f"dd_{c}{g}", tag="dd")
                    nc.sync.dma_start(out=dd[0:1, :], in_=rc[64:65, 0, :])
                    nc.sync.dma_start(out=dd[1:2, :], in_=rc[64:65, 1, :])
                    d8 = tmp.tile([8, 128], f32, name=f"d8_{c}{g}", tag="d8")
                    nc.sync.dma_start(
                        out=d8[:, :], in_=dd[:, :].rearrange("h (a b) -> (h a) b", b=128)
                    )
                    r8 = tmp.tile([8, 128], f32, name=f"r8_{c}{g}", tag="r8")
                    nc.vector.reciprocal(out=r8[:, :], in_=d8[:, :])
                    rd = rdr.tile([2, 512], f32, name=f"rd_{c}{g}", tag="rd")
                    nc.sync.dma_start(
                        out=rd[:, :].rearrange("h (a b) -> (h a) b", b=128), in_=r8[:, :]
                    )
                    bs = bsb.tile([64, 2, 512], f32, name=f"bs_{c}{g}", tag="bs")
                    for h2 in range(2):
                        rsrc = rd[h2 : h2 + 1, :]
                        bc_ap = bass.AP(
                            tensor=rsrc.tensor,
                            offset=rsrc.offset,
                            ap=[[0, 64]] + [list(d) for d in rsrc.ap[1:]],
                        )
                        nc.sync.dma_start(out=bs[:, h2, :], in_=bc_ap)
                    nc.vector.tensor_mul(
                        out=aT_sb[c][0:64, gsl], in0=at[0][0:64, :], in1=bs[:, 0, :]
                    )
                    tb = tmp.tile([64, 512], bf16, name=f"tb{c}{g}", tag="tb")
                    nc.vector.tensor_mul(out=tb, in0=at[1][0:64, :], in1=bs[:, 1, :])
                    nc.sync.dma_start(out=aT_sb[c][64:128, gsl], in_=tb)

                def emit_outproj(sl):
                    ssl = slice(sl * 128, (sl + 1) * 128)
                    pot = scr.tile([128, 2, 512], f32, name=f"po{sl}", tag=f"sc{sl % 2}")
                    for c in range(PC):
                        for jg in range(2):
                            nc.tensor.matmul(
                                pot[:, jg, :],
                                aT_sb[c][:, ssl],
                                wo_sb[:, c, jg * 512 : (jg + 1) * 512],
                                start=(c == 0),
                                stop=(c == PC - 1),
                            )
                    ot = osb.tile([128, E], bf16, name=f"ot{sl}", tag="ot")
                    if sl % 2 == 0:
                        nc.scalar.activation(
                            out=ot[:, :].rearrange("p (j e) -> p j e", j=2),
                            in_=pot[:, :, :],
                            func=mybir.ActivationFunctionType.Identity,
                        )
                    else:
                        nc.vector.tensor_copy(
                            out=ot[:, :].rearrange("p (j e) -> p j e", j=2),
                            in_=pot[:, :, :],
                        )
                    nc.sync.dma_start(out=out[ssl, :], in_=ot)

                def new_at(g, c):
                    return [
                        att.tile([65, 512], f32, name=f"at{h2}_{c}{g}", tag=f"at{h2}")
                        for h2 in range(2)
                    ]

                pending = []  # [(g, c, pts, at)]
                next_sl = 0   # out-proj tiles emitted as pipeline fills
                stage = 0
                for g in range(SGRP):
                    for c in range(PC):
                        pts = []
                        fill_v = g == 0 and c == 1
                        fill = pending[0] if (pending and not fill_v) else None
                        if fill and fill[3] is None:
                            fill = pending[0] = (fill[0], fill[1], fill[2], new_at(fill[0], fill[1]))
                        # aT for group gp is complete after norm(gp, c=1), which
                        # is emitted at the end of stage 2*gp+2 -> fill from
                        # stage 2*gp+3 on (up to 2 sl per stage)
                        sl_quota = 2 if stage >= 4 else 0
                        sl_limit = 4 * max(0, (stage - 2) // 2)
                        for tg in range(TGS):
                            emit_scores_tg(g, c, tg, pts)
                            if fill_v:
                                emit_vproj_pass(2 * tg)
                            elif fill:
                                emit_attnv_tg(fill[0], fill[1], fill[2], tg, fill[3])
                            if sl_quota > 0 and next_sl < sl_limit and tg >= 1:
                                emit_outproj(next_sl)
                                next_sl += 1
                                sl_quota -= 1
                        if fill:
                            pending.pop(0)
                            emit_norm(fill[0], fill[1], fill[3])
                        pending.append((g, c, pts, None))
                        stage += 1
                # drain
                for (g, c, pts, at) in pending:
                    if at is None:
                        at = new_at(g, c)
                    for tg in range(TGS):
                        emit_attnv_tg(g, c, pts, tg, at)
                    emit_norm(g, c, at)
                while next_sl < SQT:
                    emit_outproj(next_sl)
                    next_sl += 1



    nc.finalize()
    return nc


def _get(SK: int):
    if SK not in _cache:
        _cache[SK] = _build(SK)
    return _cache[SK]


def kernel(**inputs) -> np.ndarray:
    global LAST_EXEC_NS, LAST_RESULTS

    q = np.asarray(inputs["query"], dtype=np.float32)
    k = np.asarray(inputs["key"], dtype=np.float32)
    v = np.asarray(inputs["value"], dtype=np.float32)
    kpm = np.asarray(inputs["key_padding_mask"]).astype(bool)
    Wq = np.asarray(inputs["Wq"], dtype=np.float32)
    bq = np.asarray(inputs["bq"], dtype=np.float32)
    Wk = np.asarray(inputs["Wk"], dtype=np.float32)
    bk = np.asarray(inputs["bk"], dtype=np.float32)
    Wv = np.asarray(inputs["Wv"], dtype=np.float32)
    bv = np.asarray(inputs["bv"], dtype=np.float32)
    Wo = np.asarray(inputs["Wo"], dtype=np.float32)
    bo = np.asarray(inputs["bo"], dtype=np.float32)

    compact = not os.environ.get("KERNEL_NO_COMPACT")
    if compact:
        valid = [np.nonzero(~kpm[b])[0] for b in range(B)]
        nv = max(len(ix) for ix in valid)
        SK = max(128, ((nv + 127) // 128) * 128)
        if SK > S:
            SK = S
            compact = False
    if not compact:
        SK = S
        valid = [np.arange(S) for _ in range(B)]

    nc = _get(SK)

    # per-batch tensors
    per_b = []
    for b in range(B):
        ix = valid[b]
        n = len(ix)
        xqT = np.ascontiguousarray(q[b].T).astype(nbf16)
        kc = np.zeros((SK, E), dtype=np.float32)
        vc = np.zeros((SK, E), dtype=np.float32)
        kc[:n] = k[b][ix]
        vc[:n] = v[b][ix]
        xkT = np.ascontiguousarray(kc.T).astype(nbf16)
        xvT = np.ascontiguousarray(vc.T).astype(nbf16)
        mv = np.zeros(SK, dtype=np.float32)
        if compact:
            mv[:n] = 1.0
        else:
            mv[:] = (~kpm[b]).astype(np.float32)
        per_b.append((xqT, xkT, xvT, mv))

    ones = np.ones(64, dtype=nbf16)
    in_maps = []
    for cid in range(N_CORES):
        b, hg = cid // 4, cid % 4
        hsl = slice(hg * EL, (hg + 1) * EL)
        xqT, xkT, xvT, mv = per_b[b]
        bvh = bv[hsl].reshape(HL, 64)
        bvA = np.concatenate([bvh, np.ones((HL, 1), np.float32)], axis=1).ravel()
        in_maps.append(
            {
                "xqT": xqT,
                "xkT": xkT,
                "xvT": xvT,
                "wqT": np.ascontiguousarray((Wq[hsl] / 8.0).T).astype(nbf16),
                "wkT": np.ascontiguousarray(Wk[hsl].T).astype(nbf16),
                "wvT": np.ascontiguousarray(Wv[hsl].T).astype(nbf16),
                "woT": np.ascontiguousarray(Wo[:, hsl].T).astype(nbf16),
                "bq2": (bq[hsl] / 8.0).reshape(PC, 128),
                "bk2": bk[hsl].reshape(PC, 128),
                "bvA": bvA,
                "mk": mv,
                "ones": ones,
            }
        )

    trace = bool(os.environ.get("KERNEL_TRACE"))
    res = run_bass_kernel_spmd(
        nc, in_maps, core_ids=list(range(N_CORES)), trace=trace
    )
    LAST_EXEC_NS = res.exec_time_ns
    LAST_RESULTS = res

    out = np.empty((B, S, E), dtype=np.float32)
    for b in range(B):
        acc = res.results[b * 4]["out"].astype(np.float32)
        for hg in range(1, 4):
            acc = acc + res.results[b * 4 + hg]["out"].astype(np.float32)
        out[b] = acc + bo
    return out



# revision 11
# speedup vs baseline: 1.1069x; 1.0250x over previous
"""Multihead attention on 8 Trainium2 cores (Bass/Tile).

Sharding: core = (batch b, head-group hg); 2 batches x 4 head-groups,
4 heads per core (head dim 64, local width 256).

Per core (matmul operands bf16, PSUM accumulation fp32):
  qT = (Wq[hg]/8 @ x_q^T)          [256, 2048]   (e' on partitions)
  kT = (Wk[hg]  @ x_k^T)           [256, SK]
  v  = (x_k  @ Wv[hg]^T)           [SK, 256]     (natural layout, + ones col)
  scoresT[sk, sq] = kT^T-slices x qT  (PE, head pairs row-packed)
  probsT = exp(scoresT)            (ACT, PSUM fp32 -> SBUF bf16)
  attnT[d, sq], denom[sq] = v_aug^T @ probsT   (ones column -> denominator)
  attn = attnT / denom   (denoms repartitioned via DRAM to [8,128], exact
                          DVE reciprocal there, DRAM-bounce broadcast, mul)
  out_partial[s, :] = attn^T-chunks x Wo[:, hg]^T   (no bias on device)
Host: out[b] = sum of the 4 head-group partials + bo.
All inputs loaded upfront into persistent SBUF tiles, issue order matching
consumption (bq/bk, xq, wk, xk, wv, xv), all on the SP DMA queue.

Mask handling: the key_padding_mask (and compaction padding) zeroes v rows
AND the ones-column, so masked keys contribute exactly 0 to both the
numerator and the softmax denominator -- identical to softmax(-inf masking).

Compaction: only valid (unmasked) key positions are shipped per batch,
padded to a multiple of 128. SK is derived from the inputs at call time;
kernels are compiled once per SK and cached.
"""

import os
import sys

sys.path.insert(0, "/opt/trn_rl_repo")

import ml_dtypes
import numpy as np

import concourse.bass as bass
import concourse.mybir as mybir
import concourse.tile as tile
from concourse import bacc
from concourse.bass_utils import run_bass_kernel_spmd

B, S, E, H, D = 2, 2048, 1024, 16, 64
N_CORES = 8
HL = H // 4          # 4 heads per core
EL = HL * D          # 256 local embed width
PC = EL // 128       # 2 partition chunks of local heads
ECH = E // 128       # 8 contraction chunks for projections
SGRP = S // 512      # 4 query groups
SQT = S // 128       # 16 query tiles

f32 = mybir.dt.float32
f32r = mybir.dt.float32r
bf16 = mybir.dt.bfloat16
nbf16 = ml_dtypes.bfloat16

_cache: dict[int, object] = {}
LAST_EXEC_NS = None
LAST_RESULTS = None


def _bcast_ap(handle, shape):
    """DRAM AP broadcast along partitions: shape [128, ...dims of handle]."""
    ap = handle[:]
    dims = [[0, shape[0]]]
    # strides for the remaining dims over the flat dram tensor
    sizes = shape[1:]
    stride = 1
    rev = []
    for s in reversed(sizes):
        rev.append([stride, s])
        stride *= s
    dims.extend(reversed(rev))
    return bass.AP(tensor=ap.tensor, offset=0, ap=dims)


def _build(SK: int):
    SKT = SK // 128
    TGS = (SKT + 1) // 2
    kgroups = [(o, min(512, SK - o)) for o in range(0, SK, 512)]

    nc = bacc.Bacc(None, target_bir_lowering=False)

    xqT = nc.dram_tensor("xqT", [E, S], bf16, kind="ExternalInput")
    xkT = nc.dram_tensor("xkT", [E, SK], bf16, kind="ExternalInput")
    xvT = nc.dram_tensor("xvT", [E, SK], bf16, kind="ExternalInput")
    wqT = nc.dram_tensor("wqT", [E, EL], bf16, kind="ExternalInput")
    wkT = nc.dram_tensor("wkT", [E, EL], bf16, kind="ExternalInput")
    wvT = nc.dram_tensor("wvT", [E, EL], bf16, kind="ExternalInput")
    woT = nc.dram_tensor("woT", [EL, E], bf16, kind="ExternalInput")
    bq2 = nc.dram_tensor("bq2", [PC, 128], f32, kind="ExternalInput")
    bk2 = nc.dram_tensor("bk2", [PC, 128], f32, kind="ExternalInput")
    bvA = nc.dram_tensor("bvA", [HL * 65], f32, kind="ExternalInput")
    mk = nc.dram_tensor("mk", [SK], f32, kind="ExternalInput")
    ones = nc.dram_tensor("ones", [64], bf16, kind="ExternalInput")
    out = nc.dram_tensor("out", [S, E], bf16, kind="ExternalOutput")

    with tile.TileContext(nc) as tc, nc.allow_low_precision("fp32r attention"):
        with (
            tc.tile_pool(name="persist", bufs=1) as pp,
            tc.tile_pool(name="xs", bufs=3) as xs,
            tc.tile_pool(name="prb", bufs=3 * TGS) as prb,
            tc.tile_pool(name="rcp", bufs=2) as rcp,
            tc.tile_pool(name="bsb", bufs=2) as bsb,
            tc.tile_pool(name="tmp", bufs=2) as tmp,
            tc.tile_pool(name="osb", bufs=2) as osb,
        ):
            # ---- persistent tiles + constant loads ----
            wq_sb = pp.tile([128, ECH, EL], bf16, name="wq_sb", tag="wq_sb")
            wk_sb = pp.tile([128, ECH, EL], bf16, name="wk_sb", tag="wk_sb")
            wv_sb = pp.tile([128, ECH, EL], bf16, name="wv_sb", tag="wv_sb")
            wo_sb = pp.tile([128, PC, E], bf16, name="wo_sb", tag="wo_sb")
            nc.sync.dma_start(out=wq_sb, in_=wqT[:, :].rearrange("(c p) m -> p c m", p=128))

            bq_sb = pp.tile([128, PC], f32, name="bq_sb", tag="bq_sb")
            bk_sb = pp.tile([128, PC], f32, name="bk_sb", tag="bk_sb")

            bv_sb = pp.tile([128, HL, 65], f32, name="bv_sb", tag="bv_sb")
            nc.sync.dma_start(out=bv_sb, in_=_bcast_ap(bvA, [128, HL, 65]))
            ones_sb = pp.tile([128, 64], bf16, name="ones_sb", tag="ones_sb")
            nc.sync.dma_start(out=ones_sb, in_=_bcast_ap(ones, [128, 64]))
            m_sb = pp.tile([128, SKT], f32, name="m_sb", tag="m_sb")
            nc.sync.dma_start(out=m_sb, in_=mk[:].rearrange("(t p) -> p t", p=128))

            # upfront input loads: persistent chunk tiles, split across the
            # two hardware DMA queues (SP + Activation)
            xq_sb = pp.tile([128, ECH, S], bf16, name="xq_sb", tag="xq_sb")
            xk_sb = pp.tile([128, ECH, SK], bf16, name="xk_sb", tag="xk_sb")
            xv_sb = pp.tile([128, ECH, SK], bf16, name="xv_sb", tag="xv_sb")
            xqr = xqT[:, :].rearrange("(c p) s -> c p s", p=128)
            xkr = xkT[:, :].rearrange("(c p) s -> c p s", p=128)
            xvr = xvT[:, :].rearrange("(c p) s -> c p s", p=128)
            nc.sync.dma_start(out=bq_sb, in_=bq2[:, :].rearrange("c p -> p c"))
            nc.sync.dma_start(out=bk_sb, in_=bk2[:, :].rearrange("c p -> p c"))
            for ec in range(ECH):
                eng = nc.sync if ec % 2 == 0 else nc.scalar
                eng.dma_start(out=xq_sb[:, ec, :], in_=xqr[ec])
            nc.sync.dma_start(
                out=wk_sb, in_=wkT[:, :].rearrange("(c p) m -> p c m", p=128)
            )
            for ec in range(ECH):
                nc.sync.dma_start(out=xk_sb[:, ec, :], in_=xkr[ec])
            nc.sync.dma_start(
                out=wv_sb, in_=wvT[:, :].rearrange("(c p) m -> p c m", p=128)
            )
            for ec in range(ECH):
                nc.sync.dma_start(out=xv_sb[:, ec, :], in_=xvr[ec])

            qT_sb = [pp.tile([128, S], bf16, name=f"qT{c}", tag=f"qT{c}") for c in range(PC)]
            kT_sb = [pp.tile([128, SK], bf16, name=f"kT{c}", tag=f"kT{c}") for c in range(PC)]
            aT_sb = [pp.tile([128, S], bf16, name=f"aT{c}", tag=f"aT{c}") for c in range(PC)]
            v_sb = [
                pp.tile([128, HL, 65], bf16, name=f"v{t}", tag=f"v{t}") for t in range(SKT)
            ]

            # ---- phase 1: projections ----
            with tc.tile_pool(name="pj", bufs=1, space="PSUM") as pj:
                # q and k: out layout [e' partition, seq free]
                for (xh, slen, glist, wsb, bias_sb, dst) in (
                    (xqT, S, [(o, 512) for o in range(0, S, 512)], wq_sb, bq_sb, qT_sb),
                    (xkT, SK, kgroups, wk_sb, bk_sb, kT_sb),
                ):
                    xsb = xq_sb if xh is xqT else xk_sb
                    pqt = {}
                    for ec in range(ECH):
                        xc = xsb[:, ec, :]
                        for pc in range(PC):
                            for gi, (go, gs) in enumerate(glist):
                                idx = pc * 4 + gi
                                if ec == 0:
                                    pqt[idx] = pj.tile(
                                        [128, 512], f32, name=f"pj{idx}", tag=f"pj{idx}"
                                    )
                                nc.tensor.matmul(
                                    pqt[idx][:, 0:gs],
                                    wsb[:, ec, pc * 128 : pc * 128 + 128],
                                    xc[:, go : go + gs],
                                    start=(ec == 0),
                                    stop=(ec == ECH - 1),
                                )
                    for pc in range(PC):
                        for gi, (go, gs) in enumerate(glist):
                            idx = pc * 4 + gi
                            nc.vector.tensor_scalar_add(
                                out=dst[pc][:, go : go + gs],
                                in0=pqt[idx][:, 0:gs],
                                scalar1=bias_sb[:, pc : pc + 1],
                            )


            nc.sync.dma_start(
                out=wo_sb, in_=woT[:, :].rearrange("(c p) m -> p c m", p=128)
            )
            # ---- phase 2: software-pipelined attention ----
            # scores+exp(g,c) emitted one stage AHEAD of attnV+norm(g,c);
            # the v-projection (which waits on the xv DMA tail) is emitted
            # after the first two scores blocks so the DMA hides under them.
            with (
                tc.tile_pool(name="scr", bufs=1, space="PSUM") as scr,
                tc.tile_pool(name="att", bufs=2, space="PSUM") as att,
                tc.tile_pool(name="rdr", bufs=2, space="DRAM") as rdr,
            ):
                def emit_scores_tg(g, c, tg, pts):
                    gsl = slice(g * 512, (g + 1) * 512)
                    if True:
                        tl = [t for t in (2 * tg, 2 * tg + 1) if t < SKT]
                        nt = len(tl)
                        st = [
                            scr.tile(
                                [128, 2, 512], f32, name=f"sc{h2}_{c}{g}{tg}", tag=f"sc{h2}"
                            )
                            for h2 in range(2)
                        ]
                        for i, t in enumerate(tl):
                            for h2 in range(2):
                                hsl = slice(h2 * 64, (h2 + 1) * 64)
                                nc.tensor.matmul(
                                    st[h2][:, i, :],
                                    kT_sb[c][hsl, t * 128 : (t + 1) * 128],
                                    qT_sb[c][hsl, gsl],
                                    start=True,
                                    stop=True,
                                )
                        pt = []
                        for h2 in range(2):
                            p_ = prb.tile(
                                [128, 2, 512], bf16, name=f"pb{h2}_{c}{g}{tg}", tag=f"pb{h2}"
                            )
                            nc.scalar.activation(
                                out=p_[:, 0:nt, :],
                                in_=st[h2][:, 0:nt, :],
                                func=mybir.ActivationFunctionType.Exp,
                            )
                            pt.append(p_)
                        pts.append((tl, pt))

                def emit_vproj_pass(t0):
                    # v-projection pass (2 k-tiles) in the idle att PSUM slots
                    tl = list(range(t0, min(t0 + 2, SKT)))
                    if True:
                        pvt = {}
                        for ec in range(ECH):
                            xc = xv_sb[:, ec, t0 * 128 : t0 * 128 + len(tl) * 128]
                            for j, t in enumerate(tl):
                                if ec == 0:
                                    pvt[j] = att.tile(
                                        [128, EL], f32, name=f"pv{t}", tag=f"at{j}"
                                    )
                                nc.tensor.matmul(
                                    pvt[j][:, :],
                                    xc[:, j * 128 : (j + 1) * 128],
                                    wv_sb[:, ec, :],
                                    start=(ec == 0),
                                    stop=(ec == ECH - 1),
                                )
                        for j, t in enumerate(tl):
                            pv_view = pvt[j][:, :].rearrange("p (h d) -> p h d", h=HL)
                            vt = xs.tile([128, HL, 65], f32, name=f"vt{t}", tag="vtmp")
                            nc.vector.tensor_add(
                                out=vt[:, :, 0:64], in0=pv_view, in1=bv_sb[:, :, 0:64]
                            )
                            nc.vector.tensor_copy(
                                out=vt[:, :, 64:65], in_=bv_sb[:, :, 64:65]
                            )
                            nc.vector.tensor_scalar_mul(
                                out=v_sb[t][:, :, :],
                                in0=vt[:, :, :],
                                scalar1=m_sb[:, t : t + 1],
                            )

                def emit_attnv_tg(g, c, pts, tg, at):
                    tl, pt = pts[tg]
                    for i, t in enumerate(tl):
                        for h2 in range(2):
                            nc.tensor.matmul(
                                at[h2][:, :],
                                v_sb[t][:, 2 * c + h2, :],
                                pt[h2][:, i, :],
                                start=(t == 0),
                                stop=(t == SKT - 1),
                            )

                def emit_norm(g, c, at):
                    gsl = slice(g * 512, (g + 1) * 512)
                    # normalize: copy denom rows out of PSUM, repartition via
                    # DRAM to [8,128], exact reciprocal there (full-width DVE)
                    rc = rcp.tile([128, 2, 512], f32, name=f"rc_{c}{g}", tag="rc")
                    nc.vector.tensor_copy(out=rc[64:65, 0, :], in_=at[0][64:65, :])
                    nc.vector.tensor_copy(out=rc[64:65, 1, :], in_=at[1][64:65, :])
                    dd = rdr.tile([2, 512], f32, name=f"dd_{c}{g}", tag="dd")
                    nc.sync.dma_start(out=dd[0:1, :], in_=rc[64:65, 0, :])
                    nc.sync.dma_start(out=dd[1:2, :], in_=rc[64:65, 1, :])
                    d8 = tmp.tile([8, 128], f32, name=f"d8_{c}{g}", tag="d8")
                    nc.sync.dma_start(
                        out=d8[:, :], in_=dd[:, :].rearrange("h (a b) -> (h a) b", b=128)
                    )
                    r8 = tmp.tile([8, 128], f32, name=f"r8_{c}{g}", tag="r8")
                    nc.vector.reciprocal(out=r8[:, :], in_=d8[:, :])
                    rd = rdr.tile([2, 512], f32, name=f"rd_{c}{g}", tag="rd")
                    nc.sync.dma_start(
                        out=rd[:, :].rearrange("h (a b) -> (h a) b", b=128), in_=r8[:, :]
                    )
                    bs = bsb.tile([64, 2, 512], f32, name=f"bs_{c}{g}", tag="bs")
                    for h2 in range(2):
                        rsrc = rd[h2 : h2 + 1, :]
                        bc_ap = bass.AP(
                            tensor=rsrc.tensor,
                            offset=rsrc.offset,
                            ap=[[0, 64]] + [list(d) for d in rsrc.ap[1:]],
                        )
                        nc.sync.dma_start(out=bs[:, h2, :], in_=bc_ap)
                    nc.vector.tensor_mul(
                        out=aT_sb[c][0:64, gsl], in0=at[0][0:64, :], in1=bs[:, 0, :]
                    )
                    tb = tmp.tile([64, 512], bf16, name=f"tb{c}{g}", tag="tb")
                    nc.vector.tensor_mul(out=tb, in0=at[1][0:64, :], in1=bs[:, 1, :])
                    nc.sync.dma_start(out=aT_sb[c][64:128, gsl], in_=tb)

                def emit_outproj(sl):
                    ssl = slice(sl * 128, (sl + 1) * 128)
                    pot = scr.tile([128, 2, 512], f32, name=f"po{sl}", tag=f"sc{sl % 2}")
                    for c in range(PC):
                        for jg in range(2):
                            nc.tensor.matmul(
                                pot[:, jg, :],
                                aT_sb[c][:, ssl],
                                wo_sb[:, c, jg * 512 : (jg + 1) * 512],
                                start=(c == 0),
                                stop=(c == PC - 1),
                            )
                    ot = osb.tile([128, E], bf16, name=f"ot{sl}", tag="ot")
                    if sl % 2 == 0:
                        nc.scalar.activation(
                            out=ot[:, :].rearrange("p (j e) -> p j e", j=2),
                            in_=pot[:, :, :],
                            func=mybir.ActivationFunctionType.Identity,
                        )
                    else:
                        nc.vector.tensor_copy(
                            out=ot[:, :].rearrange("p (j e) -> p j e", j=2),
                            in_=pot[:, :, :],
                        )
                    oeng = nc.scalar if sl % 2 == 0 else nc.sync
                    oeng.dma_start(out=out[ssl, :], in_=ot)

                def new_at(g, c):
                    return [
                        att.tile([65, 512], f32, name=f"at{h2}_{c}{g}", tag=f"at{h2}")
                        for h2 in range(2)
                    ]

                pending = []  # [(g, c, pts, at)]
                next_sl = 0   # out-proj tiles emitted as pipeline fills
                stage = 0
                for g in range(SGRP):
                    for c in range(PC):
                        pts = []
                        fill_v = g == 0 and c == 1
                        fill = pending[0] if (pending and not fill_v) else None
                        if fill and fill[3] is None:
                            fill = pending[0] = (fill[0], fill[1], fill[2], new_at(fill[0], fill[1]))
                        # aT for group gp is complete after norm(gp, c=1), which
                        # is emitted at the end of stage 2*gp+2 -> fill from
                        # stage 2*gp+3 on (up to 2 sl per stage)
                        sl_quota = 2 if stage >= 4 else 0
                        sl_limit = 4 * max(0, (stage - 2) // 2)
                        for tg in range(TGS):
                            emit_scores_tg(g, c, tg, pts)
                            if fill_v:
                                emit_vproj_pass(2 * tg)
                            elif fill:
                                emit_attnv_tg(fill[0], fill[1], fill[2], tg, fill[3])
                            if sl_quota > 0 and next_sl < sl_limit and tg >= 1:
                                emit_outproj(next_sl)
                                next_sl += 1
                                sl_quota -= 1
                        if fill:
                            pending.pop(0)
                            emit_norm(fill[0], fill[1], fill[3])
                        pending.append((g, c, pts, None))
                        stage += 1
                # drain
                for (g, c, pts, at) in pending:
                    if at is None:
                        at = new_at(g, c)
                    for tg in range(TGS):
                        emit_attnv_tg(g, c, pts, tg, at)
                    emit_norm(g, c, at)
                while next_sl < SQT:
                    emit_outproj(next_sl)
                    next_sl += 1



    nc.finalize()
    return nc


def _get(SK: int):
    if SK not in _cache:
        _cache[SK] = _build(SK)
    return _cache[SK]


def kernel(**inputs) -> np.ndarray:
    global LAST_EXEC_NS, LAST_RESULTS

    q = np.asarray(inputs["query"], dtype=np.float32)
    k = np.asarray(inputs["key"], dtype=np.float32)
    v = np.asarray(inputs["value"], dtype=np.float32)
    kpm = np.asarray(inputs["key_padding_mask"]).astype(bool)
    Wq = np.asarray(inputs["Wq"], dtype=np.float32)
    bq = np.asarray(inputs["bq"], dtype=np.float32)
    Wk = np.asarray(inputs["Wk"], dtype=np.float32)
    bk = np.asarray(inputs["bk"], dtype=np.float32)
    Wv = np.asarray(inputs["Wv"], dtype=np.float32)
    bv = np.asarray(inputs["bv"], dtype=np.float32)
    Wo = np.asarray(inputs["Wo"], dtype=np.float32)
    bo = np.asarray(inputs["bo"], dtype=np.float32)

    compact = not os.environ.get("KERNEL_NO_COMPACT")
    if compact:
        valid = [np.nonzero(~kpm[b])[0] for b in range(B)]
        nv = max(len(ix) for ix in valid)
        SK = max(128, ((nv + 127) // 128) * 128)
        if SK > S:
            SK = S
            compact = False
    if not compact:
        SK = S
        valid = [np.arange(S) for _ in range(B)]

    nc = _get(SK)

    # per-batch tensors
    per_b = []
    for b in range(B):
        ix = valid[b]
        n = len(ix)
        xqT = np.ascontiguousarray(q[b].T).astype(nbf16)
        kc = np.zeros((SK, E), dtype=np.float32)
        vc = np.zeros((SK, E), dtype=np.float32)
        kc[:n] = k[b][ix]
        vc[:n] = v[b][ix]
        xkT = np.ascontiguousarray(kc.T).astype(nbf16)
        xvT = np.ascontiguousarray(vc.T).astype(nbf16)
        mv = np.zeros(SK, dtype=np.float32)
        if compact:
            mv[:n] = 1.0
        else:
            mv[:] = (~kpm[b]).astype(np.float32)
        per_b.append((xqT, xkT, xvT, mv))

    ones = np.ones(64, dtype=nbf16)
    in_maps = []
    for cid in range(N_CORES):
        b, hg = cid // 4, cid % 4
        hsl = slice(hg * EL, (hg + 1) * EL)
        xqT, xkT, xvT, mv = per_b[b]
        bvh = bv[hsl].reshape(HL, 64)
        bvA = np.concatenate([bvh, np.ones((HL, 1), np.float32)], axis=1).ravel()
        in_maps.append(
            {
                "xqT": xqT,
                "xkT": xkT,
                "xvT": xvT,
                "wqT": np.ascontiguousarray((Wq[hsl] / 8.0).T).astype(nbf16),
                "wkT": np.ascontiguousarray(Wk[hsl].T).astype(nbf16),
                "wvT": np.ascontiguousarray(Wv[hsl].T).astype(nbf16),
                "woT": np.ascontiguousarray(Wo[:, hsl].T).astype(nbf16),
                "bq2": (bq[hsl] / 8.0).reshape(PC, 128),
                "bk2": bk[hsl].reshape(PC, 128),
                "bvA": bvA,
                "mk": mv,
                "ones": ones,
            }
        )

    trace = bool(os.environ.get("KERNEL_TRACE"))
    res = run_bass_kernel_spmd(
        nc, in_maps, core_ids=list(range(N_CORES)), trace=trace
    )
    LAST_EXEC_NS = res.exec_time_ns
    LAST_RESULTS = res

    out = np.empty((B, S, E), dtype=np.float32)
    for b in range(B):
        acc = res.results[b * 4]["out"].astype(np.float32)
        for hg in range(1, 4):
            acc = acc + res.results[b * 4 + hg]["out"].astype(np.float32)
        out[b] = acc + bo
    return out

